# revision 17
# baseline (speedup 1.0000x reference)
"""Trainium2 Bass kernel for the IWE (image-warped-events) problem.

Full inputs in, full outputs out. Data-parallel over (batch, half) across 8
NeuronCores; each core computes a partial IWE grid over its events plus the
avg_flow channels; host sums the two partial IWEs per batch.

Per-core pipeline (events padded to 512000 = 250 chunks x 2048; chunk = 16
blocks x 128 partitions, e = c*2048 + n*128 + p):
  - flow gathered per event via indirect DMA from a host-staged [16384, 2]
    (fy, fx) pair table (flat index y*128+x computed on-device)
  - warped coords wy/wx in fp32 columns [128, 16]
  - hat rows: t = w - iota (stt, fp16 out), m = |t|-1 (tensor_scalar),
    hatY = Relu(-m) on ACT (positive), hatX = max(-m, 0)-style on Pool
    (positive), hatYs = min(m,0)*sgn (= -hatY*sgn) on DVE
  - scatter-add via PSUM-accumulated matmuls per 128-event block:
      accS[x, y]  += hatX^T @ hatY      (= pos+neg mass)
      accD[x, y]  += hatX^T @ hatYs     (= neg-pos mass)
    pos = (accS - accD)/2, neg = (accS + accD)/2, transposed at finalize
"""
import numpy as np

H, W = 128, 128
NCORES = 8
CHUNK = 2048                           # events per chunk
NBLK = 16                              # blocks per chunk (128 * NBLK = CHUNK)
EPC = 512000                           # padded events per core
NCHUNKS = EPC // CHUNK                 # 250

_COMPILED = {}


def _build(nchunks=NCHUNKS, use_hw_loop=True, unroll=4, passes=1):
    import concourse.bass as bass
    import concourse.bacc as bacc
    import concourse.mybir as mybir
    from concourse.tile import TileContext
    from concourse.masks import make_identity

    fp32 = mybir.dt.float32
    fp16 = mybir.dt.float16
    int32 = mybir.dt.int32
    Alu = mybir.AluOpType
    Act = mybir.ActivationFunctionType
    E = nchunks * CHUNK

    nc = bacc.Bacc("TRN2", target_bir_lowering=False, debug=False,
                   num_devices=NCORES)

    ev = nc.dram_tensor("ev", [E, 4], fp32, kind="ExternalInput").ap()
    fpair = nc.dram_tensor("fpair", [H * W, 2], fp32, kind="ExternalInput").ap()
    flow = nc.dram_tensor("flow", [2, H, W], fp32, kind="ExternalInput").ap()
    emask = nc.dram_tensor("emask", [H, W], fp32, kind="ExternalInput").ap()
    out = nc.dram_tensor("out", [4, H, W], fp32, kind="ExternalOutput").ap()

    # event e = c*CHUNK + n*128 + p  (chunk c, block n, partition p)
    ev_v = ev.rearrange("(c n p) f -> p c n f", c=nchunks, n=NBLK, p=128)

    with TileContext(nc) as tc:
        with tc.tile_pool(name="const", bufs=1) as cpool, \
             tc.tile_pool(name="work", bufs=3) as wpool, \
             tc.tile_pool(name="ppool", bufs=1, space="PSUM") as ppool:

            # ---------------- constants ----------------
            iotai = cpool.tile([128, 128], int32)
            nc.gpsimd.iota(iotai[:], pattern=[[1, 128]], base=0,
                           channel_multiplier=0)
            iota32 = cpool.tile([128, 128], fp32)
            nc.vector.tensor_copy(out=iota32[:], in_=iotai[:])
            iota16 = cpool.tile([128, 128], fp16)
            nc.vector.tensor_copy(out=iota16[:], in_=iotai[:])

            # flow rows fp32: [y, 0:128]=flow[1] (fy), [y, 128:256]=flow[0]
            flow32 = cpool.tile([128, 256], fp32)
            nc.sync.dma_start(out=flow32[:, 0:128], in_=flow[1])
            nc.sync.dma_start(out=flow32[:, 128:256], in_=flow[0])
            maskt = cpool.tile([128, 128], fp32)
            nc.sync.dma_start(out=maskt[:], in_=emask[:, :])

            # ---------------- psum scatter accumulators ----------------
            accS = ppool.tile([128, 512], fp32, tag="accS")  # [x, y] (bank)
            accD = ppool.tile([128, 512], fp32, tag="accD")

            def body(i, first=False, last=False):
                evt = wpool.tile([128, NBLK * 4], fp32, tag="evt")
                nc.sync.dma_start(out=evt[:], in_=ev_v[:, bass.ds(i, 1), :, :])
                ev3 = evt[:].rearrange("p (n f) -> p n f", f=4)
                ts4 = ev3[:, :, 0]
                y4 = ev3[:, :, 1]
                x4 = ev3[:, :, 2]
                p4 = ev3[:, :, 3]

                # ---- flow gather via indirect DMA ----
                flat = wpool.tile([128, NBLK], fp32, tag="flat")
                nc.vector.scalar_tensor_tensor(out=flat[:], in0=y4,
                                               scalar=float(W), in1=x4,
                                               op0=Alu.mult, op1=Alu.add)
                flati = wpool.tile([128, NBLK], int32, tag="flati")
                nc.vector.tensor_copy(out=flati[:], in_=flat[:])
                fyfx = wpool.tile([128, NBLK * 2], fp32, tag="fyfx")
                ff3 = fyfx[:].rearrange("p (n f) -> p n f", f=2)
                for n in range(NBLK):
                    nc.gpsimd.indirect_dma_start(
                        out=fyfx[:, 2 * n:2 * n + 2],
                        out_offset=None,
                        in_=fpair[:],
                        in_offset=bass.IndirectOffsetOnAxis(
                            ap=flati[:, n:n + 1], axis=0))
                fy4 = ff3[:, :, 0]
                fx4 = ff3[:, :, 1]

                # ---- warp (fp32 columns) ----
                u4 = wpool.tile([128, NBLK], fp32, tag="u4")
                nc.vector.tensor_scalar(out=u4[:], in0=ts4, scalar1=1.0,
                                        scalar2=-1.0, op0=Alu.subtract,
                                        op1=Alu.mult)
                sgn4 = wpool.tile([128, NBLK], fp32, tag="sgn4")
                nc.vector.tensor_scalar(out=sgn4[:], in0=p4, scalar1=2.0,
                                        scalar2=1.0, op0=Alu.mult,
                                        op1=Alu.subtract)
                wy4 = wpool.tile([128, NBLK], fp32, tag="wy4")
                wx4 = wpool.tile([128, NBLK], fp32, tag="wx4")
                nc.vector.tensor_tensor(out=wy4[:], in0=u4[:], in1=fy4,
                                        op=Alu.mult)
                nc.vector.tensor_tensor(out=wy4[:], in0=wy4[:], in1=y4,
                                        op=Alu.add)
                nc.vector.tensor_tensor(out=wx4[:], in0=u4[:], in1=fx4,
                                        op=Alu.mult)
                nc.vector.tensor_tensor(out=wx4[:], in0=wx4[:], in1=x4,
                                        op=Alu.add)

                # ---- Y hats: t = iota - wy (blockwise), a = |t| (ACT),
                # nY = min(|t|-1, 0) = -hatY, nYs = nY*sgn (Pool blockwise)
                t_y = wpool.tile([128, NBLK * 128], fp16, tag="t_y")
                ty3 = t_y[:].rearrange("p (n f) -> p n f", f=128)
                for n in range(NBLK):
                    nc.vector.tensor_scalar(out=ty3[:, n, :], in0=iota16[:],
                                            scalar1=wy4[:, n:n + 1],
                                            scalar2=None, op0=Alu.subtract)
                a_y = wpool.tile([128, NBLK * 128], fp16, tag="a_y")
                nc.scalar.activation(out=a_y[:], in_=t_y[:], func=Act.Abs)
                nY = wpool.tile([128, NBLK * 128], fp16, tag="nY")
                nc.vector.tensor_scalar(out=nY[:], in0=a_y[:], scalar1=1.0,
                                        scalar2=0.0, op0=Alu.subtract,
                                        op1=Alu.min)
                nYs = wpool.tile([128, NBLK * 128], fp16, tag="nYs")
                ny3 = nY[:].rearrange("p (n f) -> p n f", f=128)
                nys3 = nYs[:].rearrange("p (n f) -> p n f", f=128)
                for n in range(NBLK):
                    nc.gpsimd.tensor_scalar(out=nys3[:, n, :],
                                            in0=ny3[:, n, :], scalar1=0.0,
                                            scalar2=sgn4[:, n:n + 1],
                                            op0=Alu.add, op1=Alu.mult)

                # ---- X hats ----
                t_x = wpool.tile([128, NBLK * 128], fp16, tag="t_x")
                tx3 = t_x[:].rearrange("p (n f) -> p n f", f=128)
                for n in range(NBLK):
                    nc.vector.tensor_scalar(out=tx3[:, n, :], in0=iota16[:],
                                            scalar1=wx4[:, n:n + 1],
                                            scalar2=None, op0=Alu.subtract)
                a_x = wpool.tile([128, NBLK * 128], fp16, tag="a_x")
                nc.scalar.activation(out=a_x[:], in_=t_x[:], func=Act.Abs)
                nX = wpool.tile([128, NBLK * 128], fp16, tag="nX")
                nc.vector.tensor_scalar(out=nX[:], in0=a_x[:], scalar1=1.0,
                                        scalar2=0.0, op0=Alu.subtract,
                                        op1=Alu.min)

                # ---- scatter: accS += nX^T@nY (= S); accD += nX^T@nYs (= D)
                hx3 = nX[:].rearrange("p (n f) -> p n f", f=128)
                hy3 = ny3
                hys3 = nys3
                for n in range(NBLK):
                    st = first and n == 0
                    sp = last and n == NBLK - 1
                    nc.tensor.matmul(out=accS[:, 0:128], lhsT=hx3[:, n, :],
                                     rhs=hy3[:, n, :], start=st, stop=sp)
                    nc.tensor.matmul(out=accD[:, 0:128], lhsT=hx3[:, n, :],
                                     rhs=hys3[:, n, :], start=st, stop=sp)

            if use_hw_loop:
                for _ in range(passes):
                    body(0, first=True)
                    tc.For_i_unrolled(1, nchunks - 1, 1, body,
                                      max_unroll=unroll)
                    body(nchunks - 1, last=True)
            else:
                for i in range(nchunks):
                    body(i, first=(i == 0), last=(i == nchunks - 1))

            # ---------------- finalize ----------------
            ident = cpool.tile([128, 128], fp32)
            make_identity(nc, ident)
            accsb = cpool.tile([128, 256], fp32)
            nc.vector.tensor_copy(out=accsb[:, 0:128], in_=accS[:, 0:128])
            nc.vector.tensor_copy(out=accsb[:, 128:256], in_=accD[:, 0:128])
            pT = ppool.tile([128, 512], fp32, tag="pT")
            nc.tensor.transpose(out=pT[:, 0:128], in_=accsb[:, 0:128],
                                identity=ident[:])
            nc.tensor.transpose(out=pT[:, 256:384], in_=accsb[:, 128:256],
                                identity=ident[:])
            # pT[:, 0:128] = S[y, x]; pT[:, 128:256] = D[y, x]
            # pos = (S - D)/2 ; neg = (S + D)/2
            res = cpool.tile([128, 128 * 4], fp32)
            pTsb = cpool.tile([128, 128], fp32)
            nc.vector.tensor_copy(out=pTsb[:], in_=pT[:, 256:384])
            nc.vector.tensor_tensor(out=res[:, 0:128], in0=pT[:, 0:128],
                                    in1=pTsb[:], op=Alu.add)
            nc.vector.tensor_scalar_mul(out=res[:, 0:128], in0=res[:, 0:128],
                                        scalar1=0.5)
            nc.vector.tensor_tensor(out=res[:, 128:256], in0=pT[:, 0:128],
                                    in1=pTsb[:], op=Alu.subtract)
            nc.vector.tensor_scalar_mul(out=res[:, 128:256],
                                        in0=res[:, 128:256], scalar1=0.5)
            inv = 1.0 / (1.0 + 1e-9)
            nc.vector.scalar_tensor_tensor(out=res[:, 256:384],
                                           in0=flow32[:, 128:256], scalar=inv,
                                           in1=maskt[:], op0=Alu.mult,
                                           op1=Alu.mult)
            nc.vector.scalar_tensor_tensor(out=res[:, 384:512],
                                           in0=flow32[:, 0:128], scalar=inv,
                                           in1=maskt[:], op0=Alu.mult,
                                           op1=Alu.mult)
            for ch in range(4):
                nc.sync.dma_start(out=out[ch], in_=res[:, ch * 128:(ch + 1) * 128])

    nc.compile()
    return nc


def _pad_events(ev_half, total=EPC):
    """[N,4] -> [total,4]: pad with (ts=1, y=0, x=200, p=0) null events."""
    n = ev_half.shape[0]
    if n == total:
        return np.ascontiguousarray(ev_half, np.float32)
    full = np.empty((total, 4), np.float32)
    full[:n] = ev_half
    full[n:] = 0.0
    full[n:, 0] = 1.0
    full[n:, 2] = 200.0
    return full


def _run(nc, flow, event_list, pol_mask, event_mask):
    """flow [B,2,H,W], event_list [B,N,4], pol [B,N,2], emask [B,1,H,W]."""
    from concourse.bass_utils import run_bass_kernel_spmd

    Bb, Nn = event_list.shape[0], event_list.shape[1]
    half = Nn // 2
    in_maps = []
    for c in range(NCORES):
        b, h = c // 2, c % 2
        sl = slice(h * half, (h + 1) * half)
        fb = np.asarray(flow[b], np.float32)
        fpair = np.ascontiguousarray(
            np.stack([fb[1].ravel(), fb[0].ravel()], axis=1), np.float32)
        in_maps.append({
            "ev": _pad_events(np.asarray(event_list[b, sl, :], np.float32)),
            "fpair": fpair,
            "flow": np.ascontiguousarray(fb, np.float32),
            "emask": np.ascontiguousarray(event_mask[b, 0], np.float32),
        })
    res = run_bass_kernel_spmd(nc, in_maps, list(range(NCORES)))
    out = np.zeros((Bb, 4, H, W), np.float32)
    for c in range(NCORES):
        b = c // 2
        r = res.results[c]["out"]
        out[b, 0:2] += r[0:2]
        if c % 2 == 0:
            out[b, 2:4] = r[2:4]
    return out


def kernel(flow, event_list, pol_mask, event_mask):
    flow = np.asarray(flow, np.float32)
    event_list = np.asarray(event_list, np.float32)
    pol_mask = np.asarray(pol_mask, np.float32)
    event_mask = np.asarray(event_mask, np.float32)
    key = ("nc", NCHUNKS)
    if key not in _COMPILED:
        _COMPILED[key] = _build(NCHUNKS)
    return _run(_COMPILED[key], flow, event_list, pol_mask, event_mask)


# revision 18
# speedup vs baseline: 1.0070x; 1.0070x over previous
"""Trainium2 Bass kernel for the IWE (image-warped-events) problem.

Full inputs in, full outputs out. Data-parallel over (batch, half) across 8
NeuronCores; each core computes a partial IWE grid over its events plus the
avg_flow channels; host sums the two partial IWEs per batch.

Per-core pipeline (events padded to 512000 = 250 chunks x 2048; chunk = 16
blocks x 128 partitions, e = c*2048 + n*128 + p):
  - flow gathered per event via indirect DMA from a host-staged [16384, 2]
    (fy, fx) pair table (flat index y*128+x computed on-device)
  - warped coords wy/wx in fp32 columns [128, 16]
  - hat rows: t = w - iota (stt, fp16 out), m = |t|-1 (tensor_scalar),
    hatY = Relu(-m) on ACT (positive), hatX = max(-m, 0)-style on Pool
    (positive), hatYs = min(m,0)*sgn (= -hatY*sgn) on DVE
  - scatter-add via PSUM-accumulated matmuls per 128-event block:
      accS[x, y]  += hatX^T @ hatY      (= pos+neg mass)
      accD[x, y]  += hatX^T @ hatYs     (= neg-pos mass)
    pos = (accS - accD)/2, neg = (accS + accD)/2, transposed at finalize
"""
import numpy as np

H, W = 128, 128
NCORES = 8
CHUNK = 2048                           # events per chunk
NBLK = 16                              # blocks per chunk (128 * NBLK = CHUNK)
EPC = 512000                           # padded events per core
NCHUNKS = EPC // CHUNK                 # 250

_COMPILED = {}


def _build(nchunks=NCHUNKS, use_hw_loop=True, unroll=4, passes=1):
    import concourse.bass as bass
    import concourse.bacc as bacc
    import concourse.mybir as mybir
    from concourse.tile import TileContext
    from concourse.masks import make_identity

    fp32 = mybir.dt.float32
    fp16 = mybir.dt.float16
    int32 = mybir.dt.int32
    Alu = mybir.AluOpType
    Act = mybir.ActivationFunctionType
    E = nchunks * CHUNK

    nc = bacc.Bacc("TRN2", target_bir_lowering=False, debug=False,
                   num_devices=NCORES)

    ev = nc.dram_tensor("ev", [E, 4], fp32, kind="ExternalInput").ap()
    fpair = nc.dram_tensor("fpair", [H * W, 2], fp32, kind="ExternalInput").ap()
    flow = nc.dram_tensor("flow", [2, H, W], fp32, kind="ExternalInput").ap()
    emask = nc.dram_tensor("emask", [H, W], fp32, kind="ExternalInput").ap()
    out = nc.dram_tensor("out", [4, H, W], fp32, kind="ExternalOutput").ap()

    # event e = c*CHUNK + n*128 + p  (chunk c, block n, partition p)
    ev_v = ev.rearrange("(c n p) f -> p c n f", c=nchunks, n=NBLK, p=128)

    with TileContext(nc) as tc:
        with tc.tile_pool(name="const", bufs=1) as cpool, \
             tc.tile_pool(name="work", bufs=3) as wpool, \
             tc.tile_pool(name="ppool", bufs=1, space="PSUM") as ppool:

            # ---------------- constants ----------------
            iotai = cpool.tile([128, 128], int32)
            nc.gpsimd.iota(iotai[:], pattern=[[1, 128]], base=0,
                           channel_multiplier=0)
            iota32 = cpool.tile([128, 128], fp32)
            nc.vector.tensor_copy(out=iota32[:], in_=iotai[:])
            iota16 = cpool.tile([128, 128], fp16)
            nc.vector.tensor_copy(out=iota16[:], in_=iotai[:])

            # flow rows fp32: [y, 0:128]=flow[1] (fy), [y, 128:256]=flow[0]
            flow32 = cpool.tile([128, 256], fp32)
            nc.sync.dma_start(out=flow32[:, 0:128], in_=flow[1])
            nc.sync.dma_start(out=flow32[:, 128:256], in_=flow[0])
            maskt = cpool.tile([128, 128], fp32)
            nc.sync.dma_start(out=maskt[:], in_=emask[:, :])

            # ---------------- psum scatter accumulators ----------------
            accS = ppool.tile([128, 512], fp32, tag="accS")  # [x, y] (bank)
            accD = ppool.tile([128, 512], fp32, tag="accD")

            def body(i, first=False, last=False):
                evt = wpool.tile([128, NBLK * 4], fp32, tag="evt")
                nc.sync.dma_start(out=evt[:], in_=ev_v[:, bass.ds(i, 1), :, :])
                ev3 = evt[:].rearrange("p (n f) -> p n f", f=4)
                ts4 = ev3[:, :, 0]
                y4 = ev3[:, :, 1]
                x4 = ev3[:, :, 2]
                p4 = ev3[:, :, 3]

                # ---- flow gather via indirect DMA ----
                flat = wpool.tile([128, NBLK], fp32, tag="flat")
                nc.vector.scalar_tensor_tensor(out=flat[:], in0=y4,
                                               scalar=float(W), in1=x4,
                                               op0=Alu.mult, op1=Alu.add)
                flati = wpool.tile([128, NBLK], int32, tag="flati")
                nc.vector.tensor_copy(out=flati[:], in_=flat[:])
                fyfx = wpool.tile([128, NBLK * 2], fp32, tag="fyfx")
                ff3 = fyfx[:].rearrange("p (n f) -> p n f", f=2)
                for n in range(NBLK):
                    nc.gpsimd.indirect_dma_start(
                        out=fyfx[:, 2 * n:2 * n + 2],
                        out_offset=None,
                        in_=fpair[:],
                        in_offset=bass.IndirectOffsetOnAxis(
                            ap=flati[:, n:n + 1], axis=0))
                fy4 = ff3[:, :, 0]
                fx4 = ff3[:, :, 1]

                # ---- warp (fp32 columns) ----
                u4 = wpool.tile([128, NBLK], fp32, tag="u4")
                nc.vector.tensor_scalar(out=u4[:], in0=ts4, scalar1=1.0,
                                        scalar2=-1.0, op0=Alu.subtract,
                                        op1=Alu.mult)
                sgn4 = wpool.tile([128, NBLK], fp32, tag="sgn4")
                nc.vector.tensor_scalar(out=sgn4[:], in0=p4, scalar1=2.0,
                                        scalar2=1.0, op0=Alu.mult,
                                        op1=Alu.subtract)
                wy4 = wpool.tile([128, NBLK], fp32, tag="wy4")
                wx4 = wpool.tile([128, NBLK], fp32, tag="wx4")
                nc.vector.tensor_tensor(out=wy4[:], in0=u4[:], in1=fy4,
                                        op=Alu.mult)
                nc.vector.tensor_tensor(out=wy4[:], in0=wy4[:], in1=y4,
                                        op=Alu.add)
                nc.vector.tensor_tensor(out=wx4[:], in0=u4[:], in1=fx4,
                                        op=Alu.mult)
                nc.vector.tensor_tensor(out=wx4[:], in0=wx4[:], in1=x4,
                                        op=Alu.add)

                # ---- Y hats: t = iota - wy (blockwise), a = |t| (ACT),
                # nY = min(|t|-1, 0) = -hatY, nYs = nY*sgn (Pool blockwise)
                t_y = wpool.tile([128, NBLK * 128], fp16, tag="t_y")
                ty3 = t_y[:].rearrange("p (n f) -> p n f", f=128)
                for n in range(NBLK):
                    nc.vector.tensor_scalar(out=ty3[:, n, :], in0=iota16[:],
                                            scalar1=wy4[:, n:n + 1],
                                            scalar2=None, op0=Alu.subtract)
                a_y = wpool.tile([128, NBLK * 128], fp16, tag="a_y")
                nc.scalar.activation(out=a_y[:], in_=t_y[:], func=Act.Abs)
                nY = wpool.tile([128, NBLK * 128], fp16, tag="nY")
                nc.vector.tensor_scalar(out=nY[:], in0=a_y[:], scalar1=1.0,
                                        scalar2=0.0, op0=Alu.subtract,
                                        op1=Alu.min)
                nYs = wpool.tile([128, NBLK * 128], fp16, tag="nYs")
                ny3 = nY[:].rearrange("p (n f) -> p n f", f=128)
                nys3 = nYs[:].rearrange("p (n f) -> p n f", f=128)
                for n in range(NBLK):
                    nc.vector.tensor_scalar(out=nys3[:, n, :],
                                            in0=ny3[:, n, :], scalar1=0.0,
                                            scalar2=sgn4[:, n:n + 1],
                                            op0=Alu.add, op1=Alu.mult)

                # ---- X hats ----
                t_x = wpool.tile([128, NBLK * 128], fp16, tag="t_x")
                tx3 = t_x[:].rearrange("p (n f) -> p n f", f=128)
                for n in range(NBLK):
                    nc.vector.tensor_scalar(out=tx3[:, n, :], in0=iota16[:],
                                            scalar1=wx4[:, n:n + 1],
                                            scalar2=None, op0=Alu.subtract)
                a_x = wpool.tile([128, NBLK * 128], fp16, tag="a_x")
                nc.scalar.activation(out=a_x[:], in_=t_x[:], func=Act.Abs)
                nX = wpool.tile([128, NBLK * 128], fp16, tag="nX")
                nc.vector.tensor_scalar(out=nX[:], in0=a_x[:], scalar1=1.0,
                                        scalar2=0.0, op0=Alu.subtract,
                                        op1=Alu.min)

                # ---- scatter: accS += nX^T@nY (= S); accD += nX^T@nYs (= D)
                hx3 = nX[:].rearrange("p (n f) -> p n f", f=128)
                hy3 = ny3
                hys3 = nys3
                for n in range(NBLK):
                    st = first and n == 0
                    sp = last and n == NBLK - 1
                    nc.tensor.matmul(out=accS[:, 0:128], lhsT=hx3[:, n, :],
                                     rhs=hy3[:, n, :], start=st, stop=sp)
                    nc.tensor.matmul(out=accD[:, 0:128], lhsT=hx3[:, n, :],
                                     rhs=hys3[:, n, :], start=st, stop=sp)

            if use_hw_loop:
                for _ in range(passes):
                    body(0, first=True)
                    tc.For_i_unrolled(1, nchunks - 1, 1, body,
                                      max_unroll=unroll)
                    body(nchunks - 1, last=True)
            else:
                for i in range(nchunks):
                    body(i, first=(i == 0), last=(i == nchunks - 1))

            # ---------------- finalize ----------------
            ident = cpool.tile([128, 128], fp32)
            make_identity(nc, ident)
            accsb = cpool.tile([128, 256], fp32)
            nc.vector.tensor_copy(out=accsb[:, 0:128], in_=accS[:, 0:128])
            nc.vector.tensor_copy(out=accsb[:, 128:256], in_=accD[:, 0:128])
            pT = ppool.tile([128, 512], fp32, tag="pT")
            nc.tensor.transpose(out=pT[:, 0:128], in_=accsb[:, 0:128],
                                identity=ident[:])
            nc.tensor.transpose(out=pT[:, 256:384], in_=accsb[:, 128:256],
                                identity=ident[:])
            # pT[:, 0:128] = S[y, x]; pT[:, 128:256] = D[y, x]
            # pos = (S - D)/2 ; neg = (S + D)/2
            res = cpool.tile([128, 128 * 4], fp32)
            pTsb = cpool.tile([128, 128], fp32)
            nc.vector.tensor_copy(out=pTsb[:], in_=pT[:, 256:384])
            nc.vector.tensor_tensor(out=res[:, 0:128], in0=pT[:, 0:128],
                                    in1=pTsb[:], op=Alu.add)
            nc.vector.tensor_scalar_mul(out=res[:, 0:128], in0=res[:, 0:128],
                                        scalar1=0.5)
            nc.vector.tensor_tensor(out=res[:, 128:256], in0=pT[:, 0:128],
                                    in1=pTsb[:], op=Alu.subtract)
            nc.vector.tensor_scalar_mul(out=res[:, 128:256],
                                        in0=res[:, 128:256], scalar1=0.5)
            inv = 1.0 / (1.0 + 1e-9)
            nc.vector.scalar_tensor_tensor(out=res[:, 256:384],
                                           in0=flow32[:, 128:256], scalar=inv,
                                           in1=maskt[:], op0=Alu.mult,
                                           op1=Alu.mult)
            nc.vector.scalar_tensor_tensor(out=res[:, 384:512],
                                           in0=flow32[:, 0:128], scalar=inv,
                                           in1=maskt[:], op0=Alu.mult,
                                           op1=Alu.mult)
            for ch in range(4):
                nc.sync.dma_start(out=out[ch], in_=res[:, ch * 128:(ch + 1) * 128])

    nc.compile()
    return nc


def _pad_events(ev_half, total=EPC):
    """[N,4] -> [total,4]: pad with (ts=1, y=0, x=200, p=0) null events."""
    n = ev_half.shape[0]
    if n == total:
        return np.ascontiguousarray(ev_half, np.float32)
    full = np.empty((total, 4), np.float32)
    full[:n] = ev_half
    full[n:] = 0.0
    full[n:, 0] = 1.0
    full[n:, 2] = 200.0
    return full


def _run(nc, flow, event_list, pol_mask, event_mask):
    """flow [B,2,H,W], event_list [B,N,4], pol [B,N,2], emask [B,1,H,W]."""
    from concourse.bass_utils import run_bass_kernel_spmd

    Bb, Nn = event_list.shape[0], event_list.shape[1]
    half = Nn // 2
    in_maps = []
    for c in range(NCORES):
        b, h = c // 2, c % 2
        sl = slice(h * half, (h + 1) * half)
        fb = np.asarray(flow[b], np.float32)
        fpair = np.ascontiguousarray(
            np.stack([fb[1].ravel(), fb[0].ravel()], axis=1), np.float32)
        in_maps.append({
            "ev": _pad_events(np.asarray(event_list[b, sl, :], np.float32)),
            "fpair": fpair,
            "flow": np.ascontiguousarray(fb, np.float32),
            "emask": np.ascontiguousarray(event_mask[b, 0], np.float32),
        })
    res = run_bass_kernel_spmd(nc, in_maps, list(range(NCORES)))
    out = np.zeros((Bb, 4, H, W), np.float32)
    for c in range(NCORES):
        b = c // 2
        r = res.results[c]["out"]
        out[b, 0:2] += r[0:2]
        if c % 2 == 0:
            out[b, 2:4] = r[2:4]
    return out


def kernel(flow, event_list, pol_mask, event_mask):
    flow = np.asarray(flow, np.float32)
    event_list = np.asarray(event_list, np.float32)
    pol_mask = np.asarray(pol_mask, np.float32)
    event_mask = np.asarray(event_mask, np.float32)
    key = ("nc", NCHUNKS)
    if key not in _COMPILED:
        _COMPILED[key] = _build(NCHUNKS)
    return _run(_COMPILED[key], flow, event_list, pol_mask, event_mask)


# revision 20
# speedup vs baseline: 1.1856x; 1.1773x over previous
"""Trainium2 Bass kernel for the IWE (image-warped-events) problem.

Full inputs in, full outputs out. Data-parallel over (batch, half) across 8
NeuronCores; each core computes a partial IWE grid over its events plus the
avg_flow channels; host sums the two partial IWEs per batch.

Per-core pipeline (events padded to 512000 = 250 chunks x 2048; chunk = 16
blocks x 128 partitions, e = c*2048 + n*128 + p):
  - flow gathered per event via indirect DMA from a host-staged [16384, 2]
    (fy, fx) pair table (flat index y*128+x computed on-device)
  - warped coords wy/wx in fp32 columns [128, 16]
  - hat rows: t = w - iota (stt, fp16 out), m = |t|-1 (tensor_scalar),
    hatY = Relu(-m) on ACT (positive), hatX = max(-m, 0)-style on Pool
    (positive), hatYs = min(m,0)*sgn (= -hatY*sgn) on DVE
  - scatter-add via PSUM-accumulated matmuls per 128-event block:
      accS[x, y]  += hatX^T @ hatY      (= pos+neg mass)
      accD[x, y]  += hatX^T @ hatYs     (= neg-pos mass)
    pos = (accS - accD)/2, neg = (accS + accD)/2, transposed at finalize
"""
import numpy as np

H, W = 128, 128
NCORES = 8
CHUNK = 2048                           # events per chunk
NBLK = 16                              # blocks per chunk (128 * NBLK = CHUNK)
EPC = 512000                           # padded events per core
NCHUNKS = EPC // CHUNK                 # 250

_COMPILED = {}


def _build(nchunks=NCHUNKS, use_hw_loop=True, unroll=4, passes=1,
           dummy_gather=False):
    import concourse.bass as bass
    import concourse.bacc as bacc
    import concourse.mybir as mybir
    from concourse.tile import TileContext
    from concourse.masks import make_identity

    fp32 = mybir.dt.float32
    fp16 = mybir.dt.float16
    int32 = mybir.dt.int32
    Alu = mybir.AluOpType
    Act = mybir.ActivationFunctionType
    E = nchunks * CHUNK

    nc = bacc.Bacc("TRN2", target_bir_lowering=False, debug=False,
                   num_devices=NCORES)

    ev = nc.dram_tensor("ev", [E, 4], fp32, kind="ExternalInput").ap()
    fpair = nc.dram_tensor("fpair", [H * W, 2], fp32, kind="ExternalInput").ap()
    flow = nc.dram_tensor("flow", [2, H, W], fp32, kind="ExternalInput").ap()
    emask = nc.dram_tensor("emask", [H, W], fp32, kind="ExternalInput").ap()
    out = nc.dram_tensor("out", [4, H, W], fp32, kind="ExternalOutput").ap()

    # event e = c*CHUNK + n*128 + p  (chunk c, block n, partition p)
    ev_v = ev.rearrange("(c n p) f -> p c n f", c=nchunks, n=NBLK, p=128)

    with TileContext(nc) as tc:
        with tc.tile_pool(name="const", bufs=1) as cpool, \
             tc.tile_pool(name="work", bufs=3) as wpool, \
             tc.tile_pool(name="ppool", bufs=1, space="PSUM") as ppool:

            # ---------------- constants ----------------
            iotai = cpool.tile([128, 128], int32)
            nc.gpsimd.iota(iotai[:], pattern=[[1, 128]], base=0,
                           channel_multiplier=0)
            iota32 = cpool.tile([128, 128], fp32)
            nc.vector.tensor_copy(out=iota32[:], in_=iotai[:])
            iota16 = cpool.tile([128, 128], fp16)
            nc.vector.tensor_copy(out=iota16[:], in_=iotai[:])

            # flow rows fp32: [y, 0:128]=flow[1] (fy), [y, 128:256]=flow[0]
            flow32 = cpool.tile([128, 256], fp32)
            nc.sync.dma_start(out=flow32[:, 0:128], in_=flow[1])
            nc.sync.dma_start(out=flow32[:, 128:256], in_=flow[0])
            maskt = cpool.tile([128, 128], fp32)
            nc.sync.dma_start(out=maskt[:], in_=emask[:, :])

            # ---------------- psum scatter accumulators ----------------
            accS = ppool.tile([128, 512], fp32, tag="accS")  # [x, y] (bank)
            accD = ppool.tile([128, 512], fp32, tag="accD")

            def body(i, first=False, last=False):
                evt = wpool.tile([128, NBLK * 4], fp32, tag="evt")
                nc.sync.dma_start(out=evt[:], in_=ev_v[:, bass.ds(i, 1), :, :])
                ev3 = evt[:].rearrange("p (n f) -> p n f", f=4)
                ts4 = ev3[:, :, 0]
                y4 = ev3[:, :, 1]
                x4 = ev3[:, :, 2]
                p4 = ev3[:, :, 3]

                # ---- flow gather via indirect DMA ----
                flat = wpool.tile([128, NBLK], fp32, tag="flat")
                nc.vector.scalar_tensor_tensor(out=flat[:], in0=y4,
                                               scalar=float(W), in1=x4,
                                               op0=Alu.mult, op1=Alu.add)
                flati = wpool.tile([128, NBLK], int32, tag="flati")
                nc.vector.tensor_copy(out=flati[:], in_=flat[:])
                fyfx = wpool.tile([128, NBLK * 2], fp32, tag="fyfx")
                ff3 = fyfx[:].rearrange("p (n f) -> p n f", f=2)
                if dummy_gather:
                    nc.vector.tensor_copy(out=fyfx[:], in_=evt[:, 0:NBLK * 2])
                else:
                    for n in range(NBLK):
                        nc.gpsimd.indirect_dma_start(
                            out=fyfx[:, 2 * n:2 * n + 2],
                            out_offset=None,
                            in_=fpair[:],
                            in_offset=bass.IndirectOffsetOnAxis(
                                ap=flati[:, n:n + 1], axis=0))
                fy4 = ff3[:, :, 0]
                fx4 = ff3[:, :, 1]

                # ---- warp (fp32 columns) ----
                u4 = wpool.tile([128, NBLK], fp32, tag="u4")
                nc.vector.tensor_scalar(out=u4[:], in0=ts4, scalar1=1.0,
                                        scalar2=-1.0, op0=Alu.subtract,
                                        op1=Alu.mult)
                sgn16 = wpool.tile([128, NBLK], fp16, tag="sgn16")
                nc.vector.tensor_scalar(out=sgn16[:], in0=p4, scalar1=2.0,
                                        scalar2=1.0, op0=Alu.mult,
                                        op1=Alu.subtract)
                wy4 = wpool.tile([128, NBLK], fp32, tag="wy4")
                wx4 = wpool.tile([128, NBLK], fp32, tag="wx4")
                nc.vector.tensor_tensor(out=wy4[:], in0=u4[:], in1=fy4,
                                        op=Alu.mult)
                nc.vector.tensor_tensor(out=wy4[:], in0=wy4[:], in1=y4,
                                        op=Alu.add)
                nc.vector.tensor_tensor(out=wx4[:], in0=u4[:], in1=fx4,
                                        op=Alu.mult)
                nc.vector.tensor_tensor(out=wx4[:], in0=wx4[:], in1=x4,
                                        op=Alu.add)

                # ---- Y hats: t = wy - iota (one stt), a = |t| (ACT),
                # nY = min(|t|-1, 0) = -hatY, nYs = nY*sgn (one stt)
                iota_rep = iota32[:].unsqueeze(1).broadcast_to([128, NBLK, 128])
                t_y = wpool.tile([128, NBLK * 128], fp16, tag="t_y")
                wy_bc = wy4[:].unsqueeze(2).broadcast_to([128, NBLK, 128])
                nc.vector.scalar_tensor_tensor(
                    out=t_y[:].rearrange("p (n f) -> p n f", f=128),
                    in0=wy_bc, scalar=0.0, in1=iota_rep,
                    op0=Alu.add, op1=Alu.subtract)
                a_y = wpool.tile([128, NBLK * 128], fp16, tag="a_y")
                nc.scalar.activation(out=a_y[:], in_=t_y[:], func=Act.Abs)
                nY = wpool.tile([128, NBLK * 128], fp16, tag="nY")
                nc.vector.tensor_scalar(out=nY[:], in0=a_y[:], scalar1=1.0,
                                        scalar2=0.0, op0=Alu.subtract,
                                        op1=Alu.min)
                nYs = wpool.tile([128, NBLK * 128], fp16, tag="nYs")
                ny3 = nY[:].rearrange("p (n f) -> p n f", f=128)
                nys3 = nYs[:].rearrange("p (n f) -> p n f", f=128)
                sgn_bc = sgn16[:].unsqueeze(2).broadcast_to([128, NBLK, 128])
                nc.vector.scalar_tensor_tensor(
                    out=nys3, in0=ny3, scalar=0.0, in1=sgn_bc,
                    op0=Alu.add, op1=Alu.mult)

                # ---- X hats ----
                t_x = wpool.tile([128, NBLK * 128], fp16, tag="t_x")
                wx_bc = wx4[:].unsqueeze(2).broadcast_to([128, NBLK, 128])
                nc.vector.scalar_tensor_tensor(
                    out=t_x[:].rearrange("p (n f) -> p n f", f=128),
                    in0=wx_bc, scalar=0.0, in1=iota_rep,
                    op0=Alu.add, op1=Alu.subtract)
                a_x = wpool.tile([128, NBLK * 128], fp16, tag="a_x")
                nc.scalar.activation(out=a_x[:], in_=t_x[:], func=Act.Abs)
                nX = wpool.tile([128, NBLK * 128], fp16, tag="nX")
                nc.vector.tensor_scalar(out=nX[:], in0=a_x[:], scalar1=1.0,
                                        scalar2=0.0, op0=Alu.subtract,
                                        op1=Alu.min)

                # ---- scatter: accS += nX^T@nY (= S); accD += nX^T@nYs (= D)
                hx3 = nX[:].rearrange("p (n f) -> p n f", f=128)
                hy3 = ny3
                hys3 = nys3
                for n in range(NBLK):
                    st = first and n == 0
                    sp = last and n == NBLK - 1
                    nc.tensor.matmul(out=accS[:, 0:128], lhsT=hx3[:, n, :],
                                     rhs=hy3[:, n, :], start=st, stop=sp)
                    nc.tensor.matmul(out=accD[:, 0:128], lhsT=hx3[:, n, :],
                                     rhs=hys3[:, n, :], start=st, stop=sp)

            if use_hw_loop:
                for _ in range(passes):
                    body(0, first=True)
                    tc.For_i_unrolled(1, nchunks - 1, 1, body,
                                      max_unroll=unroll)
                    body(nchunks - 1, last=True)
            else:
                for i in range(nchunks):
                    body(i, first=(i == 0), last=(i == nchunks - 1))

            # ---------------- finalize ----------------
            ident = cpool.tile([128, 128], fp32)
            make_identity(nc, ident)
            accsb = cpool.tile([128, 256], fp32)
            nc.vector.tensor_copy(out=accsb[:, 0:128], in_=accS[:, 0:128])
            nc.vector.tensor_copy(out=accsb[:, 128:256], in_=accD[:, 0:128])
            pT = ppool.tile([128, 512], fp32, tag="pT")
            nc.tensor.transpose(out=pT[:, 0:128], in_=accsb[:, 0:128],
                                identity=ident[:])
            nc.tensor.transpose(out=pT[:, 256:384], in_=accsb[:, 128:256],
                                identity=ident[:])
            # pT[:, 0:128] = S[y, x]; pT[:, 128:256] = D[y, x]
            # pos = (S - D)/2 ; neg = (S + D)/2
            res = cpool.tile([128, 128 * 4], fp32)
            pTsb = cpool.tile([128, 128], fp32)
            nc.vector.tensor_copy(out=pTsb[:], in_=pT[:, 256:384])
            nc.vector.tensor_tensor(out=res[:, 0:128], in0=pT[:, 0:128],
                                    in1=pTsb[:], op=Alu.add)
            nc.vector.tensor_scalar_mul(out=res[:, 0:128], in0=res[:, 0:128],
                                        scalar1=0.5)
            nc.vector.tensor_tensor(out=res[:, 128:256], in0=pT[:, 0:128],
                                    in1=pTsb[:], op=Alu.subtract)
            nc.vector.tensor_scalar_mul(out=res[:, 128:256],
                                        in0=res[:, 128:256], scalar1=0.5)
            inv = 1.0 / (1.0 + 1e-9)
            nc.vector.scalar_tensor_tensor(out=res[:, 256:384],
                                           in0=flow32[:, 128:256], scalar=inv,
                                           in1=maskt[:], op0=Alu.mult,
                                           op1=Alu.mult)
            nc.vector.scalar_tensor_tensor(out=res[:, 384:512],
                                           in0=flow32[:, 0:128], scalar=inv,
                                           in1=maskt[:], op0=Alu.mult,
                                           op1=Alu.mult)
            for ch in range(4):
                nc.sync.dma_start(out=out[ch], in_=res[:, ch * 128:(ch + 1) * 128])

    nc.compile()
    return nc


def _pad_events(ev_half, total=EPC):
    """[N,4] -> [total,4]: pad with (ts=1, y=0, x=200, p=0) null events."""
    n = ev_half.shape[0]
    if n == total:
        return np.ascontiguousarray(ev_half, np.float32)
    full = np.empty((total, 4), np.float32)
    full[:n] = ev_half
    full[n:] = 0.0
    full[n:, 0] = 1.0
    full[n:, 2] = 200.0
    return full


def _run(nc, flow, event_list, pol_mask, event_mask):
    """flow [B,2,H,W], event_list [B,N,4], pol [B,N,2], emask [B,1,H,W]."""
    from concourse.bass_utils import run_bass_kernel_spmd

    Bb, Nn = event_list.shape[0], event_list.shape[1]
    half = Nn // 2
    in_maps = []
    for c in range(NCORES):
        b, h = c // 2, c % 2
        sl = slice(h * half, (h + 1) * half)
        fb = np.asarray(flow[b], np.float32)
        fpair = np.ascontiguousarray(
            np.stack([fb[1].ravel(), fb[0].ravel()], axis=1), np.float32)
        in_maps.append({
            "ev": _pad_events(np.asarray(event_list[b, sl, :], np.float32)),
            "fpair": fpair,
            "flow": np.ascontiguousarray(fb, np.float32),
            "emask": np.ascontiguousarray(event_mask[b, 0], np.float32),
        })
    res = run_bass_kernel_spmd(nc, in_maps, list(range(NCORES)))
    out = np.zeros((Bb, 4, H, W), np.float32)
    for c in range(NCORES):
        b = c // 2
        r = res.results[c]["out"]
        out[b, 0:2] += r[0:2]
        if c % 2 == 0:
            out[b, 2:4] = r[2:4]
    return out


def kernel(flow, event_list, pol_mask, event_mask):
    flow = np.asarray(flow, np.float32)
    event_list = np.asarray(event_list, np.float32)
    pol_mask = np.asarray(pol_mask, np.float32)
    event_mask = np.asarray(event_mask, np.float32)
    key = ("nc", NCHUNKS)
    if key not in _COMPILED:
        _COMPILED[key] = _build(NCHUNKS)
    return _run(_COMPILED[key], flow, event_list, pol_mask, event_mask)


# revision 21
# speedup vs baseline: 1.3822x; 1.1658x over previous
"""Trainium2 Bass kernel for the IWE (image-warped-events) problem.

Full inputs in, full outputs out. Data-parallel over (batch, half) across 8
NeuronCores; each core computes a partial IWE grid over its events plus the
avg_flow channels; host sums the two partial IWEs per batch.

Per-core pipeline (events padded to 512000 = 250 chunks x 2048; chunk = 16
blocks x 128 partitions, e = c*2048 + n*128 + p):
  - flow gathered per event via indirect DMA from a host-staged [16384, 2]
    (fy, fx) pair table (flat index y*128+x computed on-device)
  - warped coords wy/wx in fp32 columns [128, 16]
  - hat rows: t = w - iota (stt, fp16 out), m = |t|-1 (tensor_scalar),
    hatY = Relu(-m) on ACT (positive), hatX = max(-m, 0)-style on Pool
    (positive), hatYs = min(m,0)*sgn (= -hatY*sgn) on DVE
  - scatter-add via PSUM-accumulated matmuls per 128-event block:
      accS[x, y]  += hatX^T @ hatY      (= pos+neg mass)
      accD[x, y]  += hatX^T @ hatYs     (= neg-pos mass)
    pos = (accS - accD)/2, neg = (accS + accD)/2, transposed at finalize
"""
import numpy as np

H, W = 128, 128
NCORES = 8
CHUNK = 2048                           # events per chunk
NBLK = 16                              # blocks per chunk (128 * NBLK = CHUNK)
EPC = 512000                           # padded events per core
NCHUNKS = EPC // CHUNK                 # 250

_COMPILED = {}


def _build(nchunks=NCHUNKS, use_hw_loop=True, unroll=4, passes=1,
           dummy_gather=False):
    import concourse.bass as bass
    import concourse.bacc as bacc
    import concourse.mybir as mybir
    from concourse.tile import TileContext
    from concourse.masks import make_identity

    fp32 = mybir.dt.float32
    fp16 = mybir.dt.float16
    int32 = mybir.dt.int32
    Alu = mybir.AluOpType
    Act = mybir.ActivationFunctionType
    E = nchunks * CHUNK

    nc = bacc.Bacc("TRN2", target_bir_lowering=False, debug=False,
                   num_devices=NCORES)

    ev = nc.dram_tensor("ev", [E, 4], fp32, kind="ExternalInput").ap()
    fpair = nc.dram_tensor("fpair", [H * W, 2], fp32, kind="ExternalInput").ap()
    flow = nc.dram_tensor("flow", [2, H, W], fp32, kind="ExternalInput").ap()
    emask = nc.dram_tensor("emask", [H, W], fp32, kind="ExternalInput").ap()
    out = nc.dram_tensor("out", [4, H, W], fp32, kind="ExternalOutput").ap()

    # event e = c*CHUNK + n*128 + p  (chunk c, block n, partition p)
    ev_v = ev.rearrange("(c n p) f -> p c n f", c=nchunks, n=NBLK, p=128)

    with TileContext(nc) as tc:
        with tc.tile_pool(name="const", bufs=1) as cpool, \
             tc.tile_pool(name="work", bufs=3) as wpool, \
             tc.tile_pool(name="ppool", bufs=1, space="PSUM") as ppool:

            # ---------------- constants ----------------
            iotai = cpool.tile([128, 128], int32)
            nc.gpsimd.iota(iotai[:], pattern=[[1, 128]], base=0,
                           channel_multiplier=0)
            iota32 = cpool.tile([128, 128], fp32)
            nc.vector.tensor_copy(out=iota32[:], in_=iotai[:])
            iota16 = cpool.tile([128, 128], fp16)
            nc.vector.tensor_copy(out=iota16[:], in_=iotai[:])

            # flow rows fp32: [y, 0:128]=flow[1] (fy), [y, 128:256]=flow[0]
            flow32 = cpool.tile([128, 256], fp32)
            nc.sync.dma_start(out=flow32[:, 0:128], in_=flow[1])
            nc.sync.dma_start(out=flow32[:, 128:256], in_=flow[0])
            maskt = cpool.tile([128, 128], fp32)
            nc.sync.dma_start(out=maskt[:], in_=emask[:, :])

            # ---------------- psum scatter accumulators ----------------
            accSD = ppool.tile([128, 512], fp32, tag="accSD")  # [x, yS|yD]

            def body(i, first=False, last=False):
                evt = wpool.tile([128, NBLK * 4], fp32, tag="evt")
                nc.sync.dma_start(out=evt[:], in_=ev_v[:, bass.ds(i, 1), :, :])
                ev3 = evt[:].rearrange("p (n f) -> p n f", f=4)
                ts4 = ev3[:, :, 0]
                y4 = ev3[:, :, 1]
                x4 = ev3[:, :, 2]
                p4 = ev3[:, :, 3]

                # ---- flow gather via indirect DMA ----
                flat = wpool.tile([128, NBLK], fp32, tag="flat")
                nc.vector.scalar_tensor_tensor(out=flat[:], in0=y4,
                                               scalar=float(W), in1=x4,
                                               op0=Alu.mult, op1=Alu.add)
                flati = wpool.tile([128, NBLK], int32, tag="flati")
                nc.vector.tensor_copy(out=flati[:], in_=flat[:])
                fyfx = wpool.tile([128, NBLK * 2], fp32, tag="fyfx")
                ff3 = fyfx[:].rearrange("p (n f) -> p n f", f=2)
                if dummy_gather:
                    nc.vector.tensor_copy(out=fyfx[:], in_=evt[:, 0:NBLK * 2])
                else:
                    for n in range(NBLK):
                        nc.gpsimd.indirect_dma_start(
                            out=fyfx[:, 2 * n:2 * n + 2],
                            out_offset=None,
                            in_=fpair[:],
                            in_offset=bass.IndirectOffsetOnAxis(
                                ap=flati[:, n:n + 1], axis=0))
                fy4 = ff3[:, :, 0]
                fx4 = ff3[:, :, 1]

                # ---- warp (fp32 columns) ----
                u4 = wpool.tile([128, NBLK], fp32, tag="u4")
                nc.vector.tensor_scalar(out=u4[:], in0=ts4, scalar1=1.0,
                                        scalar2=-1.0, op0=Alu.subtract,
                                        op1=Alu.mult)
                sgn16 = wpool.tile([128, NBLK], fp16, tag="sgn16")
                nc.vector.tensor_scalar(out=sgn16[:], in0=p4, scalar1=2.0,
                                        scalar2=1.0, op0=Alu.mult,
                                        op1=Alu.subtract)
                wy4 = wpool.tile([128, NBLK], fp32, tag="wy4")
                wx4 = wpool.tile([128, NBLK], fp32, tag="wx4")
                nc.vector.tensor_tensor(out=wy4[:], in0=u4[:], in1=fy4,
                                        op=Alu.mult)
                nc.vector.tensor_tensor(out=wy4[:], in0=wy4[:], in1=y4,
                                        op=Alu.add)
                nc.vector.tensor_tensor(out=wx4[:], in0=u4[:], in1=fx4,
                                        op=Alu.mult)
                nc.vector.tensor_tensor(out=wx4[:], in0=wx4[:], in1=x4,
                                        op=Alu.add)

                # ---- Y hats: t = wy - iota (one stt), a = |t| (ACT),
                # nY = min(|t|-1, 0) = -hatY, nYs = nY*sgn (one stt)
                iota_rep = iota32[:].unsqueeze(1).broadcast_to([128, NBLK, 128])
                t_y = wpool.tile([128, NBLK * 128], fp16, tag="t_y")
                wy_bc = wy4[:].unsqueeze(2).broadcast_to([128, NBLK, 128])
                nc.vector.scalar_tensor_tensor(
                    out=t_y[:].rearrange("p (n f) -> p n f", f=128),
                    in0=wy_bc, scalar=0.0, in1=iota_rep,
                    op0=Alu.add, op1=Alu.subtract)
                a_y = wpool.tile([128, NBLK * 128], fp16, tag="a_y")
                nc.scalar.activation(out=a_y[:], in_=t_y[:], func=Act.Abs)
                comb = wpool.tile([128, NBLK * 256], fp16, tag="comb")
                comb4 = comb[:].rearrange("p (n c f) -> p n c f", c=2, f=128)
                ny3 = comb4[:, :, 0, :]
                nys3 = comb4[:, :, 1, :]
                nc.vector.tensor_scalar(out=ny3, in0=a_y[:].rearrange(
                    "p (n f) -> p n f", f=128), scalar1=1.0,
                    scalar2=0.0, op0=Alu.subtract, op1=Alu.min)
                sgn_bc = sgn16[:].unsqueeze(2).broadcast_to([128, NBLK, 128])
                nc.vector.scalar_tensor_tensor(
                    out=nys3, in0=ny3, scalar=0.0, in1=sgn_bc,
                    op0=Alu.add, op1=Alu.mult)

                # ---- X hats ----
                t_x = wpool.tile([128, NBLK * 128], fp16, tag="t_x")
                wx_bc = wx4[:].unsqueeze(2).broadcast_to([128, NBLK, 128])
                nc.vector.scalar_tensor_tensor(
                    out=t_x[:].rearrange("p (n f) -> p n f", f=128),
                    in0=wx_bc, scalar=0.0, in1=iota_rep,
                    op0=Alu.add, op1=Alu.subtract)
                a_x = wpool.tile([128, NBLK * 128], fp16, tag="a_x")
                nc.scalar.activation(out=a_x[:], in_=t_x[:], func=Act.Abs)
                nX = wpool.tile([128, NBLK * 128], fp16, tag="nX")
                nc.vector.tensor_scalar(out=nX[:], in0=a_x[:], scalar1=1.0,
                                        scalar2=0.0, op0=Alu.subtract,
                                        op1=Alu.min)

                # ---- scatter: accS += nX^T@nY (= S); accD += nX^T@nYs (= D)
                hx3 = nX[:].rearrange("p (n f) -> p n f", f=128)
                for n in range(NBLK):
                    st = first and n == 0
                    sp = last and n == NBLK - 1
                    nc.tensor.matmul(out=accSD[:, 0:256], lhsT=hx3[:, n, :],
                                     rhs=comb4[:, n, :, :], start=st, stop=sp)

            if use_hw_loop:
                for _ in range(passes):
                    body(0, first=True)
                    tc.For_i_unrolled(1, nchunks - 1, 1, body,
                                      max_unroll=unroll)
                    body(nchunks - 1, last=True)
            else:
                for i in range(nchunks):
                    body(i, first=(i == 0), last=(i == nchunks - 1))

            # ---------------- finalize ----------------
            ident = cpool.tile([128, 128], fp32)
            make_identity(nc, ident)
            accsb = cpool.tile([128, 256], fp32)
            nc.vector.tensor_copy(out=accsb[:], in_=accSD[:, 0:256])
            pT = ppool.tile([128, 512], fp32, tag="pT")
            nc.tensor.transpose(out=pT[:, 0:128], in_=accsb[:, 0:128],
                                identity=ident[:])
            nc.tensor.transpose(out=pT[:, 256:384], in_=accsb[:, 128:256],
                                identity=ident[:])
            # pT[:, 0:128] = S[y, x]; pT[:, 128:256] = D[y, x]
            # pos = (S - D)/2 ; neg = (S + D)/2
            res = cpool.tile([128, 128 * 4], fp32)
            pTsb = cpool.tile([128, 128], fp32)
            nc.vector.tensor_copy(out=pTsb[:], in_=pT[:, 256:384])
            nc.vector.tensor_tensor(out=res[:, 0:128], in0=pT[:, 0:128],
                                    in1=pTsb[:], op=Alu.add)
            nc.vector.tensor_scalar_mul(out=res[:, 0:128], in0=res[:, 0:128],
                                        scalar1=0.5)
            nc.vector.tensor_tensor(out=res[:, 128:256], in0=pT[:, 0:128],
                                    in1=pTsb[:], op=Alu.subtract)
            nc.vector.tensor_scalar_mul(out=res[:, 128:256],
                                        in0=res[:, 128:256], scalar1=0.5)
            inv = 1.0 / (1.0 + 1e-9)
            nc.vector.scalar_tensor_tensor(out=res[:, 256:384],
                                           in0=flow32[:, 128:256], scalar=inv,
                                           in1=maskt[:], op0=Alu.mult,
                                           op1=Alu.mult)
            nc.vector.scalar_tensor_tensor(out=res[:, 384:512],
                                           in0=flow32[:, 0:128], scalar=inv,
                                           in1=maskt[:], op0=Alu.mult,
                                           op1=Alu.mult)
            for ch in range(4):
                nc.sync.dma_start(out=out[ch], in_=res[:, ch * 128:(ch + 1) * 128])

    nc.compile()
    return nc


def _pad_events(ev_half, total=EPC):
    """[N,4] -> [total,4]: pad with (ts=1, y=0, x=200, p=0) null events."""
    n = ev_half.shape[0]
    if n == total:
        return np.ascontiguousarray(ev_half, np.float32)
    full = np.empty((total, 4), np.float32)
    full[:n] = ev_half
    full[n:] = 0.0
    full[n:, 0] = 1.0
    full[n:, 2] = 200.0
    return full


def _run(nc, flow, event_list, pol_mask, event_mask):
    """flow [B,2,H,W], event_list [B,N,4], pol [B,N,2], emask [B,1,H,W]."""
    from concourse.bass_utils import run_bass_kernel_spmd

    Bb, Nn = event_list.shape[0], event_list.shape[1]
    half = Nn // 2
    in_maps = []
    for c in range(NCORES):
        b, h = c // 2, c % 2
        sl = slice(h * half, (h + 1) * half)
        fb = np.asarray(flow[b], np.float32)
        fpair = np.ascontiguousarray(
            np.stack([fb[1].ravel(), fb[0].ravel()], axis=1), np.float32)
        in_maps.append({
            "ev": _pad_events(np.asarray(event_list[b, sl, :], np.float32)),
            "fpair": fpair,
            "flow": np.ascontiguousarray(fb, np.float32),
            "emask": np.ascontiguousarray(event_mask[b, 0], np.float32),
        })
    res = run_bass_kernel_spmd(nc, in_maps, list(range(NCORES)))
    out = np.zeros((Bb, 4, H, W), np.float32)
    for c in range(NCORES):
        b = c // 2
        r = res.results[c]["out"]
        out[b, 0:2] += r[0:2]
        if c % 2 == 0:
            out[b, 2:4] = r[2:4]
    return out


def kernel(flow, event_list, pol_mask, event_mask):
    flow = np.asarray(flow, np.float32)
    event_list = np.asarray(event_list, np.float32)
    pol_mask = np.asarray(pol_mask, np.float32)
    event_mask = np.asarray(event_mask, np.float32)
    key = ("nc", NCHUNKS)
    if key not in _COMPILED:
        _COMPILED[key] = _build(NCHUNKS)
    return _run(_COMPILED[key], flow, event_list, pol_mask, event_mask)
